# revision 9
# baseline (speedup 1.0000x reference)
"""Trainium2 Bass kernel for the CIR Euler-Maruyama sampling problem.

Full inputs:  x (16384, 64, 1) f32, W (16384, 2048) f32, kappa/mu/sigma (1,) f32
Full output:  (16384, 2048, 1) f32

Strategy: pure data-parallel over batch across 8 NeuronCores (2048 rows/core).
Rows are laid out as [128 partitions x 16 free]; W / output are passed to the
device pre-transposed to time-major [128, S, 16] so every on-chip access and
DMA is contiguous.

Per-step recurrence v' = v + kappa*(m - v)*dt + sigma*sqrt(relu(v)*dt)*w is
computed as (everything fp32):
    y   = max(v, 0)            [DVE tensor_tensor, chain]
    sq  = Sqrt(y * (sigma^2*dt))  [ACT, immediate scale, chain]
    t6  = sq * w_t             [DVE tensor_tensor, chain]
    v'  = ubar + t6            [DVE tensor_tensor, chain -> V history]
    av  = A * v'               [DVE, shadow op during ACT latency]
    ubar'= av + mprime         [DVE, shadow]   (ubar = a*v + kdt*m)
Dependent tensor_tensor ops pipeline at ~87ns on the DVE; the
DVE->ACT->DVE sqrt round-trip (~600ns) dominates the 2048-step chain.
Post-processing out = 0.5*v + 0.5*xmean runs on GPSIMD per chunk; DMA on the
sync engine overlaps with compute.
"""

import numpy as np
from contextlib import ExitStack

import concourse.bass as bass
import concourse.bacc as bacc
import concourse.tile as tile
import concourse.mybir as mybir
from concourse.bass_utils import run_bass_kernel_spmd

F32 = mybir.dt.float32
AF = mybir.ActivationFunctionType
OP = mybir.AluOpType
AX = mybir.AxisListType

N_CORES = 8
B_FULL = 16384
S_FULL = 2048
L = 64
P = 128
B_CORE = B_FULL // N_CORES  # 2048
G = B_CORE // P             # 16 row-groups in the free dim

_prog_cache = {}


def _build(s_len, tc_steps, sig2dt):
    """Build + compile the per-core Bass program. sig2dt is baked as an
    immediate into the Sqrt activation's scale."""
    assert s_len % tc_steps == 0
    nchunk = s_len // tc_steps

    nc = bacc.Bacc("TRN2", target_bir_lowering=False, debug=False)

    xdr = nc.dram_tensor("x_in", [P, G, L], F32, kind="ExternalInput")
    wdr = nc.dram_tensor("w_in", [P, s_len, G], F32, kind="ExternalInput")
    scdr = nc.dram_tensor("sc_in", [P, 4], F32, kind="ExternalInput")
    odr = nc.dram_tensor("out", [P, s_len, G], F32, kind="ExternalOutput")

    with ExitStack() as ctx:
        tc = ctx.enter_context(tile.TileContext(nc))
        const = ctx.enter_context(tc.tile_pool(name="const", bufs=1))
        wpool = ctx.enter_context(tc.tile_pool(name="wpool", bufs=2))
        vpool = ctx.enter_context(tc.tile_pool(name="vpool", bufs=2))
        opool = ctx.enter_context(tc.tile_pool(name="opool", bufs=2))
        smalls = ctx.enter_context(tc.tile_pool(name="smalls", bufs=4))

        # ---- prologue: constants ----
        x_sb = const.tile([P, G, L], F32, tag="x_sb")
        nc.sync.dma_start(out=x_sb[:], in_=xdr.ap())
        sc_sb = const.tile([P, 4], F32, tag="sc_sb")
        nc.sync.dma_start(out=sc_sb[:], in_=scdr.ap())
        kdt_pp = sc_sb[:, 0:1]   # kappa*dt
        a_pp = sc_sb[:, 1:2]     # 1 - kappa*dt
        mu_pp = sc_sb[:, 2:3]    # mu

        xmr = const.tile([P, G], F32, tag="xmr")
        nc.vector.tensor_reduce(xmr[:], x_sb[:], axis=AX.X, op=OP.add)
        m = const.tile([P, G], F32, tag="m")
        nc.vector.tensor_scalar(m[:], xmr[:], 1.0 / L, mu_pp, OP.mult, OP.add)
        xm2 = const.tile([P, G], F32, tag="xm2")
        nc.vector.tensor_scalar(xm2[:], xmr[:], 0.5 / L, None, OP.mult)
        mprime = const.tile([P, G], F32, tag="mprime")
        nc.vector.tensor_scalar(mprime[:], m[:], kdt_pp, None, OP.mult)
        zero = const.tile([P, G], F32, tag="zero")
        nc.vector.memset(zero[:], 0.0)
        v0t = const.tile([P, G], F32, tag="v0")
        nc.vector.memset(v0t[:], 0.04)
        # ubar_0 = a*v0 + mprime
        ubar = const.tile([P, G], F32, tag="ubar")
        nc.vector.scalar_tensor_tensor(
            ubar[:], v0t[:], a_pp, mprime[:], OP.mult, OP.add
        )

        # ---- main recurrence ----
        v_prev = v0t[:, :]
        for c in range(nchunk):
            wk = wpool.tile([P, tc_steps, G], F32, tag="wk")
            nc.sync.dma_start(
                out=wk[:], in_=wdr.ap()[:, c * tc_steps:(c + 1) * tc_steps, :]
            )
            vk = vpool.tile([P, tc_steps, G], F32, tag="vk")
            for tau in range(tc_steps):
                y = smalls.tile([P, G], F32, tag="y")
                sq = smalls.tile([P, G], F32, tag="sq")
                t6 = smalls.tile([P, G], F32, tag="t6")
                nc.vector.tensor_max(y[:], v_prev, zero[:])
                nc.scalar.activation(sq[:], y[:], AF.Sqrt, bias=0.0, scale=sig2dt)
                nc.vector.tensor_mul(t6[:], sq[:], wk[:, tau, :])
                v_new = vk[:, tau, :]
                nc.vector.tensor_add(v_new, ubar[:], t6[:])
                # shadow op for next step's ubar (hidden under ACT latency)
                ubar = smalls.tile([P, G], F32, tag="ubar_l")
                nc.vector.scalar_tensor_tensor(
                    ubar[:], v_new, a_pp, mprime[:], OP.mult, OP.add
                )
                v_prev = v_new

            ok = opool.tile([P, tc_steps, G], F32, tag="ok")
            for g in range(G):
                nc.scalar.activation(
                    ok[:, :, g], vk[:, :, g], AF.Identity,
                    bias=xm2[:, g:g + 1], scale=0.5,
                )
            nc.sync.dma_start(
                out=odr.ap()[:, c * tc_steps:(c + 1) * tc_steps, :], in_=ok[:]
            )

    nc.compile()
    return nc


def _get_prog(sig2dt, s_len=S_FULL, tc_steps=256):
    key = (s_len, tc_steps, float(sig2dt))
    if key not in _prog_cache:
        _prog_cache[key] = _build(s_len, tc_steps, float(sig2dt))
    return _prog_cache[key]


def _make_sc(kappa, mu):
    dt = np.float32(1.0 / S_FULL)
    kdt = np.float32(np.float32(kappa) * dt)
    sc = np.empty((P, 4), np.float32)
    sc[:, 0] = kdt
    sc[:, 1] = np.float32(np.float32(1.0) - kdt)
    sc[:, 2] = np.float32(mu)
    sc[:, 3] = 0.0
    return sc


def _pretranspose_w(w_core, s_len):
    # (2048, S) row-major -> [P, S, G] time-major: out[p, t, g] = w[g*128+p, t]
    return np.ascontiguousarray(
        w_core.reshape(G, P, s_len).transpose(1, 2, 0)
    )


def _pretranspose_x(x_core):
    return np.ascontiguousarray(x_core.reshape(G, P, L).transpose(1, 0, 2))


def _untranspose_out(o_core, s_len):
    # [P, S, G] -> (2048, S)
    return o_core.transpose(2, 0, 1).reshape(B_CORE, s_len)


def kernel(x, W, kappa, mu, sigma, _trace=False):
    x = np.ascontiguousarray(np.asarray(x, np.float32).reshape(B_FULL, L))
    W = np.ascontiguousarray(np.asarray(W, np.float32))
    kappa_v = float(np.asarray(kappa).reshape(-1)[0])
    mu_v = float(np.asarray(mu).reshape(-1)[0])
    sigma_v = np.float32(np.asarray(sigma).reshape(-1)[0])
    dt = np.float32(1.0 / S_FULL)
    sig2dt = np.float32(np.float32(sigma_v * sigma_v) * dt)
    sc = _make_sc(kappa_v, mu_v)

    nc = _get_prog(sig2dt)
    in_maps = []
    for i in range(N_CORES):
        sl = slice(i * B_CORE, (i + 1) * B_CORE)
        in_maps.append({
            "x_in": _pretranspose_x(x[sl]),
            "w_in": _pretranspose_w(W[sl], S_FULL),
            "sc_in": sc,
        })

    res = run_bass_kernel_spmd(nc, in_maps, list(range(N_CORES)), trace=_trace)
    out = np.concatenate(
        [_untranspose_out(r["out"], S_FULL) for r in res.results], axis=0
    )
    out = out.reshape(B_FULL, S_FULL, 1).astype(np.float32)
    if _trace:
        return out, res
    return out


# revision 12
# speedup vs baseline: 1.1513x; 1.1513x over previous
"""Trainium2 Bass kernel for the CIR Euler-Maruyama sampling problem.

Full inputs:  x (16384, 64, 1) f32, W (16384, 2048) f32, kappa/mu/sigma (1,) f32
Full output:  (16384, 2048, 1) f32

Strategy: pure data-parallel over batch across 8 NeuronCores (2048 rows/core).
Rows are laid out as [128 partitions x 16 free]; W / output are passed to the
device pre-transposed to time-major [128, S, 16] so every on-chip access and
DMA is contiguous.

Per-step recurrence v' = v + kappa*(m - v)*dt + sigma*sqrt(relu(v)*dt)*w is
computed as (everything fp32):
    y   = max(v, 0)            [DVE tensor_tensor, chain]
    sq  = Sqrt(y * (sigma^2*dt))  [ACT, immediate scale, chain]
    t6  = sq * w_t             [DVE tensor_tensor, chain]
    v'  = ubar + t6            [DVE tensor_tensor, chain -> V history]
    av  = A * v'               [DVE, shadow op during ACT latency]
    ubar'= av + mprime         [DVE, shadow]   (ubar = a*v + kdt*m)
Dependent tensor_tensor ops pipeline at ~87ns on the DVE; the
DVE->ACT->DVE sqrt round-trip (~600ns) dominates the 2048-step chain.
Post-processing out = 0.5*v + 0.5*xmean runs on GPSIMD per chunk; DMA on the
sync engine overlaps with compute.
"""

import numpy as np
from contextlib import ExitStack

import concourse.bass as bass
import concourse.bacc as bacc
import concourse.tile as tile
import concourse.mybir as mybir
from concourse.bass_utils import run_bass_kernel_spmd

F32 = mybir.dt.float32
AF = mybir.ActivationFunctionType
OP = mybir.AluOpType
AX = mybir.AxisListType

N_CORES = 8
B_FULL = 16384
S_FULL = 2048
L = 64
P = 128
B_CORE = B_FULL // N_CORES  # 2048
G = B_CORE // P             # 16 row-groups in the free dim

_prog_cache = {}


def _build(s_len, tc_steps, sig2dt):
    """Build + compile the per-core Bass program. sig2dt is baked as an
    immediate into the Sqrt activation's scale."""
    assert s_len % tc_steps == 0
    nchunk = s_len // tc_steps

    nc = bacc.Bacc("TRN2", target_bir_lowering=False, debug=False)

    xdr = nc.dram_tensor("x_in", [P, G, L], F32, kind="ExternalInput")
    # W arrives zero-interleaved: [..., 2g] = 0, [..., 2g+1] = w  (scan data0)
    wdr = nc.dram_tensor("w_in", [P, s_len, 2 * G], F32, kind="ExternalInput")
    scdr = nc.dram_tensor("sc_in", [P, 4], F32, kind="ExternalInput")
    odr = nc.dram_tensor("out", [P, s_len, G], F32, kind="ExternalOutput")

    with ExitStack() as ctx:
        tc = ctx.enter_context(tile.TileContext(nc))
        const = ctx.enter_context(tc.tile_pool(name="const", bufs=1))
        wpool = ctx.enter_context(tc.tile_pool(name="wpool", bufs=2))
        vpool = ctx.enter_context(tc.tile_pool(name="vpool", bufs=2))
        opool = ctx.enter_context(tc.tile_pool(name="opool", bufs=2))
        smalls = ctx.enter_context(tc.tile_pool(name="smalls", bufs=4))

        # ---- prologue: constants ----
        x_sb = const.tile([P, G, L], F32, tag="x_sb")
        nc.sync.dma_start(out=x_sb[:], in_=xdr.ap())
        sc_sb = const.tile([P, 4], F32, tag="sc_sb")
        nc.sync.dma_start(out=sc_sb[:], in_=scdr.ap())
        kdt_pp = sc_sb[:, 0:1]   # kappa*dt
        a_pp = sc_sb[:, 1:2]     # 1 - kappa*dt
        mu_pp = sc_sb[:, 2:3]    # mu

        xmr = const.tile([P, G], F32, tag="xmr")
        nc.vector.tensor_reduce(xmr[:], x_sb[:], axis=AX.X, op=OP.add)
        m = const.tile([P, G], F32, tag="m")
        nc.vector.tensor_scalar(m[:], xmr[:], 1.0 / L, mu_pp, OP.mult, OP.add)
        xm2 = const.tile([P, G], F32, tag="xm2")
        nc.vector.tensor_scalar(xm2[:], xmr[:], 0.5 / L, None, OP.mult)
        mprime = const.tile([P, G], F32, tag="mprime")
        nc.vector.tensor_scalar(mprime[:], m[:], kdt_pp, None, OP.mult)
        zero = const.tile([P, G], F32, tag="zero")
        nc.vector.memset(zero[:], 0.0)
        v0t = const.tile([P, G], F32, tag="v0")
        nc.vector.memset(v0t[:], 0.04)
        # first scan pair: [sq_0, ubar_0]
        dcur = smalls.tile([P, G, 2], F32, tag="dpair")
        nc.scalar.activation(
            dcur[:, :, 0], v0t[:], AF.Sqrt, bias=0.0, scale=sig2dt
        )
        nc.vector.scalar_tensor_tensor(
            dcur[:, :, 1], v0t[:], a_pp, mprime[:], OP.mult, OP.add
        )

        # ---- main recurrence ----
        # per step one scan over pairs (0,sq),(w,ubar):
        #   j0: state = 0*state + sq            -> sq
        #   j1: state = w*sq + ubar             -> v'
        for c in range(nchunk):
            wk = wpool.tile([P, tc_steps, 2 * G], F32, tag="wk")
            nc.sync.dma_start(
                out=wk[:], in_=wdr.ap()[:, c * tc_steps:(c + 1) * tc_steps, :]
            )
            vk = vpool.tile([P, tc_steps, G, 2], F32, tag="vk")
            for tau in range(tc_steps):
                vpair = vk[:, tau, :, :].rearrange("p g t -> p (g t)")
                nc.vector.tensor_tensor_scan(
                    vpair, wk[:, tau, :],
                    dcur[:, :, :].rearrange("p g t -> p (g t)"),
                    0.0, OP.mult, OP.add,
                )
                v_new = vk[:, tau, :, 1]
                dnext = smalls.tile([P, G, 2], F32, tag="dpair")
                # shadow op: next ubar (hidden under ACT latency)
                nc.vector.scalar_tensor_tensor(
                    dnext[:, :, 1], v_new, a_pp, mprime[:], OP.mult, OP.add
                )
                y = smalls.tile([P, G], F32, tag="y")
                nc.vector.tensor_max(y[:], v_new, zero[:])
                nc.scalar.activation(
                    dnext[:, :, 0], y[:], AF.Sqrt, bias=0.0, scale=sig2dt
                )
                dcur = dnext

            ok = opool.tile([P, tc_steps, G], F32, tag="ok")
            for g in range(G):
                nc.scalar.activation(
                    ok[:, :, g], vk[:, :, g, 1], AF.Identity,
                    bias=xm2[:, g:g + 1], scale=0.5,
                )
            nc.sync.dma_start(
                out=odr.ap()[:, c * tc_steps:(c + 1) * tc_steps, :], in_=ok[:]
            )

    nc.compile()
    return nc


def _get_prog(sig2dt, s_len=S_FULL, tc_steps=256):
    key = (s_len, tc_steps, float(sig2dt))
    if key not in _prog_cache:
        _prog_cache[key] = _build(s_len, tc_steps, float(sig2dt))
    return _prog_cache[key]


def _make_sc(kappa, mu):
    dt = np.float32(1.0 / S_FULL)
    kdt = np.float32(np.float32(kappa) * dt)
    sc = np.empty((P, 4), np.float32)
    sc[:, 0] = kdt
    sc[:, 1] = np.float32(np.float32(1.0) - kdt)
    sc[:, 2] = np.float32(mu)
    sc[:, 3] = 0.0
    return sc


def _pretranspose_w(w_core, s_len):
    # (2048, S) row-major -> zero-interleaved time-major [P, S, 2G]:
    # out[p, t, 2g] = 0, out[p, t, 2g+1] = w[g*128+p, t]  (scan data0)
    wt = w_core.reshape(G, P, s_len).transpose(1, 2, 0)
    wz = np.zeros((P, s_len, 2 * G), np.float32)
    wz[:, :, 1::2] = wt
    return wz


def _pretranspose_x(x_core):
    return np.ascontiguousarray(x_core.reshape(G, P, L).transpose(1, 0, 2))


def _untranspose_out(o_core, s_len):
    # [P, S, G] -> (2048, S)
    return o_core.transpose(2, 0, 1).reshape(B_CORE, s_len)


def kernel(x, W, kappa, mu, sigma, _trace=False):
    x = np.ascontiguousarray(np.asarray(x, np.float32).reshape(B_FULL, L))
    W = np.ascontiguousarray(np.asarray(W, np.float32))
    kappa_v = float(np.asarray(kappa).reshape(-1)[0])
    mu_v = float(np.asarray(mu).reshape(-1)[0])
    sigma_v = np.float32(np.asarray(sigma).reshape(-1)[0])
    dt = np.float32(1.0 / S_FULL)
    sig2dt = np.float32(np.float32(sigma_v * sigma_v) * dt)
    sc = _make_sc(kappa_v, mu_v)

    nc = _get_prog(sig2dt)
    in_maps = []
    for i in range(N_CORES):
        sl = slice(i * B_CORE, (i + 1) * B_CORE)
        in_maps.append({
            "x_in": _pretranspose_x(x[sl]),
            "w_in": _pretranspose_w(W[sl], S_FULL),
            "sc_in": sc,
        })

    res = run_bass_kernel_spmd(nc, in_maps, list(range(N_CORES)), trace=_trace)
    out = np.concatenate(
        [_untranspose_out(r["out"], S_FULL) for r in res.results], axis=0
    )
    out = out.reshape(B_FULL, S_FULL, 1).astype(np.float32)
    if _trace:
        return out, res
    return out


# revision 13
# speedup vs baseline: 1.1673x; 1.0139x over previous
"""Trainium2 Bass kernel for the CIR Euler-Maruyama sampling problem.

Full inputs:  x (16384, 64, 1) f32, W (16384, 2048) f32, kappa/mu/sigma (1,) f32
Full output:  (16384, 2048, 1) f32

Strategy: pure data-parallel over batch across 8 NeuronCores (2048 rows/core).
Rows are laid out as [128 partitions x 16 free]; W / output are passed to the
device pre-transposed to time-major [128, S, 16] so every on-chip access and
DMA is contiguous.

Per-step recurrence v' = v + kappa*(m - v)*dt + sigma*sqrt(relu(v)*dt)*w is
computed as (everything fp32):
    y   = max(v, 0)            [DVE tensor_tensor, chain]
    sq  = Sqrt(y * (sigma^2*dt))  [ACT, immediate scale, chain]
    t6  = sq * w_t             [DVE tensor_tensor, chain]
    v'  = ubar + t6            [DVE tensor_tensor, chain -> V history]
    av  = A * v'               [DVE, shadow op during ACT latency]
    ubar'= av + mprime         [DVE, shadow]   (ubar = a*v + kdt*m)
Dependent tensor_tensor ops pipeline at ~87ns on the DVE; the
DVE->ACT->DVE sqrt round-trip (~600ns) dominates the 2048-step chain.
Post-processing out = 0.5*v + 0.5*xmean runs on GPSIMD per chunk; DMA on the
sync engine overlaps with compute.
"""

import numpy as np
from contextlib import ExitStack

import concourse.bass as bass
import concourse.bacc as bacc
import concourse.tile as tile
import concourse.mybir as mybir
from concourse.bass_utils import run_bass_kernel_spmd

F32 = mybir.dt.float32
AF = mybir.ActivationFunctionType
OP = mybir.AluOpType
AX = mybir.AxisListType

N_CORES = 8
B_FULL = 16384
S_FULL = 2048
L = 64
P = 128
B_CORE = B_FULL // N_CORES  # 2048
G = B_CORE // P             # 16 row-groups in the free dim

_prog_cache = {}


def _build(s_len, tc_steps, sig2dt):
    """Build + compile the per-core Bass program. sig2dt is baked as an
    immediate into the Sqrt activation's scale."""
    assert s_len % tc_steps == 0
    nchunk = s_len // tc_steps

    nc = bacc.Bacc("TRN2", target_bir_lowering=False, debug=False)

    xdr = nc.dram_tensor("x_in", [P, G, L], F32, kind="ExternalInput")
    # W arrives zero-interleaved: [..., 2g] = 0, [..., 2g+1] = w  (scan data0)
    wdr = nc.dram_tensor("w_in", [P, s_len, 2 * G], F32, kind="ExternalInput")
    scdr = nc.dram_tensor("sc_in", [P, 4], F32, kind="ExternalInput")
    odr = nc.dram_tensor("out", [P, s_len, G], F32, kind="ExternalOutput")

    with ExitStack() as ctx:
        tc = ctx.enter_context(tile.TileContext(nc))
        const = ctx.enter_context(tc.tile_pool(name="const", bufs=1))
        wpool = ctx.enter_context(tc.tile_pool(name="wpool", bufs=2))
        vpool = ctx.enter_context(tc.tile_pool(name="vpool", bufs=2))
        opool = ctx.enter_context(tc.tile_pool(name="opool", bufs=2))
        smalls = ctx.enter_context(tc.tile_pool(name="smalls", bufs=4))

        # ---- prologue: constants ----
        x_sb = const.tile([P, G, L], F32, tag="x_sb")
        nc.sync.dma_start(out=x_sb[:], in_=xdr.ap())
        sc_sb = const.tile([P, 4], F32, tag="sc_sb")
        nc.sync.dma_start(out=sc_sb[:], in_=scdr.ap())
        kdt_pp = sc_sb[:, 0:1]   # kappa*dt
        a_pp = sc_sb[:, 1:2]     # 1 - kappa*dt
        mu_pp = sc_sb[:, 2:3]    # mu

        xmr = const.tile([P, G], F32, tag="xmr")
        nc.vector.tensor_reduce(xmr[:], x_sb[:], axis=AX.X, op=OP.add)
        m = const.tile([P, G], F32, tag="m")
        nc.vector.tensor_scalar(m[:], xmr[:], 1.0 / L, mu_pp, OP.mult, OP.add)
        xm2 = const.tile([P, G], F32, tag="xm2")
        nc.vector.tensor_scalar(xm2[:], xmr[:], 0.5 / L, None, OP.mult)
        mprime = const.tile([P, G], F32, tag="mprime")
        nc.vector.tensor_scalar(mprime[:], m[:], kdt_pp, None, OP.mult)
        zero = const.tile([P, G], F32, tag="zero")
        nc.vector.memset(zero[:], 0.0)
        v0t = const.tile([P, G], F32, tag="v0")
        nc.vector.memset(v0t[:], 0.04)
        # first scan pair: [sq_0, ubar_0]
        dcur = smalls.tile([P, G, 2], F32, tag="dpair")
        nc.scalar.activation(
            dcur[:, :, 0], v0t[:], AF.Sqrt, bias=0.0, scale=sig2dt
        )
        nc.vector.scalar_tensor_tensor(
            dcur[:, :, 1], v0t[:], a_pp, mprime[:], OP.mult, OP.add
        )

        # ---- main recurrence ----
        # per step one scan over pairs (0,sq),(w,ubar):
        #   j0: state = 0*state + sq            -> sq
        #   j1: state = w*sq + ubar             -> v'
        for c in range(nchunk):
            wk = wpool.tile([P, tc_steps, 2 * G], F32, tag="wk")
            nc.sync.dma_start(
                out=wk[:], in_=wdr.ap()[:, c * tc_steps:(c + 1) * tc_steps, :]
            )
            vk = vpool.tile([P, tc_steps, G, 2], F32, tag="vk")
            for tau in range(tc_steps):
                vpair = vk[:, tau, :, :].rearrange("p g t -> p (g t)")
                nc.vector.tensor_tensor_scan(
                    vpair, wk[:, tau, :],
                    dcur[:, :, :].rearrange("p g t -> p (g t)"),
                    0.0, OP.mult, OP.add,
                )
                v_new = vk[:, tau, :, 1]
                dnext = smalls.tile([P, G, 2], F32, tag="dpair")
                # shadow op: next ubar (hidden under ACT latency)
                nc.vector.scalar_tensor_tensor(
                    dnext[:, :, 1], v_new, a_pp, mprime[:], OP.mult, OP.add
                )
                y = smalls.tile([P, G], F32, tag="y")
                nc.vector.tensor_max(y[:], v_new, zero[:])
                nc.scalar.activation(
                    dnext[:, :, 0], y[:], AF.Sqrt, bias=0.0, scale=sig2dt
                )
                dcur = dnext

            ok = opool.tile([P, tc_steps, G], F32, tag="ok")
            for g in range(G):
                nc.vector.tensor_scalar(
                    ok[:, :, g], vk[:, :, g, 1], 0.5, xm2[:, g:g + 1],
                    OP.mult, OP.add,
                )
            nc.sync.dma_start(
                out=odr.ap()[:, c * tc_steps:(c + 1) * tc_steps, :], in_=ok[:]
            )

    nc.compile()
    return nc


def _get_prog(sig2dt, s_len=S_FULL, tc_steps=256):
    key = (s_len, tc_steps, float(sig2dt))
    if key not in _prog_cache:
        _prog_cache[key] = _build(s_len, tc_steps, float(sig2dt))
    return _prog_cache[key]


def _make_sc(kappa, mu):
    dt = np.float32(1.0 / S_FULL)
    kdt = np.float32(np.float32(kappa) * dt)
    sc = np.empty((P, 4), np.float32)
    sc[:, 0] = kdt
    sc[:, 1] = np.float32(np.float32(1.0) - kdt)
    sc[:, 2] = np.float32(mu)
    sc[:, 3] = 0.0
    return sc


def _pretranspose_w(w_core, s_len):
    # (2048, S) row-major -> zero-interleaved time-major [P, S, 2G]:
    # out[p, t, 2g] = 0, out[p, t, 2g+1] = w[g*128+p, t]  (scan data0)
    wt = w_core.reshape(G, P, s_len).transpose(1, 2, 0)
    wz = np.zeros((P, s_len, 2 * G), np.float32)
    wz[:, :, 1::2] = wt
    return wz


def _pretranspose_x(x_core):
    return np.ascontiguousarray(x_core.reshape(G, P, L).transpose(1, 0, 2))


def _untranspose_out(o_core, s_len):
    # [P, S, G] -> (2048, S)
    return o_core.transpose(2, 0, 1).reshape(B_CORE, s_len)


def kernel(x, W, kappa, mu, sigma, _trace=False):
    x = np.ascontiguousarray(np.asarray(x, np.float32).reshape(B_FULL, L))
    W = np.ascontiguousarray(np.asarray(W, np.float32))
    kappa_v = float(np.asarray(kappa).reshape(-1)[0])
    mu_v = float(np.asarray(mu).reshape(-1)[0])
    sigma_v = np.float32(np.asarray(sigma).reshape(-1)[0])
    dt = np.float32(1.0 / S_FULL)
    sig2dt = np.float32(np.float32(sigma_v * sigma_v) * dt)
    sc = _make_sc(kappa_v, mu_v)

    nc = _get_prog(sig2dt)
    in_maps = []
    for i in range(N_CORES):
        sl = slice(i * B_CORE, (i + 1) * B_CORE)
        in_maps.append({
            "x_in": _pretranspose_x(x[sl]),
            "w_in": _pretranspose_w(W[sl], S_FULL),
            "sc_in": sc,
        })

    res = run_bass_kernel_spmd(nc, in_maps, list(range(N_CORES)), trace=_trace)
    out = np.concatenate(
        [_untranspose_out(r["out"], S_FULL) for r in res.results], axis=0
    )
    out = out.reshape(B_FULL, S_FULL, 1).astype(np.float32)
    if _trace:
        return out, res
    return out


# revision 20
# speedup vs baseline: 1.3536x; 1.1596x over previous
"""Trainium2 Bass kernel for the CIR Euler-Maruyama sampling problem.

Full inputs:  x (16384, 64, 1) f32, W (16384, 2048) f32, kappa/mu/sigma (1,) f32
Full output:  (16384, 2048, 1) f32

Strategy: pure data-parallel over batch across 8 NeuronCores (2048 rows/core).
Rows are laid out as [128 partitions x 16 free]; W / output are passed to the
device pre-transposed to time-major [128, S, 16] so every on-chip access and
DMA is contiguous.

Per-step recurrence v' = v + kappa*(m - v)*dt + sigma*sqrt(relu(v)*dt)*w is
computed as (everything fp32), with ubar = a*v + kappa*dt*m, a = 1-kappa*dt:
    sq  = Sqrt(y * (sigma^2*dt))   [ACT, immediate scale, on chain]
    v'  = scan pair: (0*st+sq), (w*sq+ubar)  [one DVE tensor_tensor_scan,
                                              W arrives zero-interleaved]
    ubar'= (v' * a) + mprime       [DVE stt, hidden under ACT latency]
    y   = max(v', 0)               [DVE tensor_tensor, on chain -> next sqrt]
The DVE->ACT->DVE sqrt round-trip plus two DVE links (~1.05us/step) dominates
the 2048-step sequential chain; per-chunk post-processing
out = 0.5*v + 0.5*xmean runs on DVE in the idle windows while waiting on ACT,
and all DMA (time-major contiguous, host-pretransposed) overlaps on the sync
engine.
"""

import numpy as np
from contextlib import ExitStack

import concourse.bass as bass
import concourse.bacc as bacc
import concourse.tile as tile
import concourse.mybir as mybir
from concourse.bass_utils import run_bass_kernel_spmd

F32 = mybir.dt.float32
AF = mybir.ActivationFunctionType
OP = mybir.AluOpType
AX = mybir.AxisListType

N_CORES = 8
B_FULL = 16384
S_FULL = 2048
L = 64
P = 128
B_CORE = B_FULL // N_CORES  # 2048
G = B_CORE // P             # 16 row-groups in the free dim

_prog_cache = {}


def _build(s_len, tc_steps, sig2dt):
    """Build + compile the per-core Bass program. sig2dt is baked as an
    immediate into the Sqrt activation's scale."""
    assert s_len % tc_steps == 0
    nchunk = s_len // tc_steps

    nc = bacc.Bacc("TRN2", target_bir_lowering=False, debug=False)

    xdr = nc.dram_tensor("x_in", [P, G, L], F32, kind="ExternalInput")
    # W arrives zero-interleaved: [..., 2g] = 0, [..., 2g+1] = w  (scan data0)
    wdr = nc.dram_tensor("w_in", [P, s_len, 2 * G], F32, kind="ExternalInput")
    scdr = nc.dram_tensor("sc_in", [P, 4], F32, kind="ExternalInput")
    odr = nc.dram_tensor("out", [P, s_len, G], F32, kind="ExternalOutput")

    with ExitStack() as ctx:
        tc = ctx.enter_context(tile.TileContext(nc))
        const = ctx.enter_context(tc.tile_pool(name="const", bufs=1))
        wpool = ctx.enter_context(tc.tile_pool(name="wpool", bufs=2))
        vpool = ctx.enter_context(tc.tile_pool(name="vpool", bufs=2))
        opool = ctx.enter_context(tc.tile_pool(name="opool", bufs=2))
        smalls = ctx.enter_context(tc.tile_pool(name="smalls", bufs=8))

        # ---- prologue: constants ----
        x_sb = const.tile([P, G, L], F32, tag="x_sb")
        nc.sync.dma_start(out=x_sb[:], in_=xdr.ap())
        sc_sb = const.tile([P, 4], F32, tag="sc_sb")
        nc.sync.dma_start(out=sc_sb[:], in_=scdr.ap())
        kdt_pp = sc_sb[:, 0:1]   # kappa*dt
        a_pp = sc_sb[:, 1:2]     # 1 - kappa*dt
        mu_pp = sc_sb[:, 2:3]    # mu

        xmr = const.tile([P, G], F32, tag="xmr")
        nc.vector.tensor_reduce(xmr[:], x_sb[:], axis=AX.X, op=OP.add)
        m = const.tile([P, G], F32, tag="m")
        nc.vector.tensor_scalar(m[:], xmr[:], 1.0 / L, mu_pp, OP.mult, OP.add)
        xm2 = const.tile([P, G], F32, tag="xm2")
        nc.vector.tensor_scalar(xm2[:], xmr[:], 0.5 / L, None, OP.mult)
        mprime = const.tile([P, G], F32, tag="mprime")
        nc.vector.tensor_scalar(mprime[:], m[:], kdt_pp, None, OP.mult)
        zero = const.tile([P, G], F32, tag="zero")
        nc.vector.memset(zero[:], 0.0)
        v0t = const.tile([P, G], F32, tag="v0")
        nc.vector.memset(v0t[:], 0.04)
        # first scan pair: [sq_0, ubar_0]
        dcur = smalls.tile([P, G, 2], F32, tag="dpair")
        nc.scalar.activation(
            dcur[:, :, 0], v0t[:], AF.Sqrt, bias=0.0, scale=sig2dt
        )
        nc.vector.scalar_tensor_tensor(
            dcur[:, :, 1], v0t[:], a_pp, mprime[:], OP.mult, OP.add
        )

        # ---- main recurrence ----
        # per step one scan over pairs (0,sq),(w,ubar):
        #   j0: state = 0*state + sq            -> sq
        #   j1: state = w*sq + ubar             -> v'
        for c in range(nchunk):
            wk = wpool.tile([P, tc_steps, 2 * G], F32, tag="wk")
            nc.sync.dma_start(
                out=wk[:], in_=wdr.ap()[:, c * tc_steps:(c + 1) * tc_steps, :]
            )
            vk = vpool.tile([P, tc_steps, G, 2], F32, tag="vk")
            for tau in range(tc_steps):
                vpair = vk[:, tau, :, :].rearrange("p g t -> p (g t)")
                nc.vector.tensor_tensor_scan(
                    vpair, wk[:, tau, :],
                    dcur[:, :, :].rearrange("p g t -> p (g t)"),
                    0.0, OP.mult, OP.add,
                )
                v_new = vk[:, tau, :, 1]
                dnext = smalls.tile([P, G, 2], F32, tag="dpair")
                # y first: it is chain-critical (feeds the next sqrt); the
                # ubar shadow op trails behind it in the in-order DVE queue.
                y = smalls.tile([P, G], F32, tag="y")
                nc.vector.tensor_scalar(y[:], v_new, 0.0, None, OP.max)
                nc.vector.scalar_tensor_tensor(
                    dnext[:, :, 1], v_new, a_pp, mprime[:], OP.mult, OP.add
                )
                nc.scalar.activation(
                    dnext[:, :, 0], y[:], AF.Sqrt, bias=0.0, scale=sig2dt
                )
                dcur = dnext

            ok = opool.tile([P, tc_steps, G], F32, tag="ok")
            for g in range(G):
                nc.vector.tensor_scalar(
                    ok[:, :, g], vk[:, :, g, 1], 0.5, xm2[:, g:g + 1],
                    OP.mult, OP.add,
                )
            nc.sync.dma_start(
                out=odr.ap()[:, c * tc_steps:(c + 1) * tc_steps, :], in_=ok[:]
            )

    nc.compile()
    return nc


def _get_prog(sig2dt, s_len=S_FULL, tc_steps=256):
    key = (s_len, tc_steps, float(sig2dt))
    if key not in _prog_cache:
        _prog_cache[key] = _build(s_len, tc_steps, float(sig2dt))
    return _prog_cache[key]


def _make_sc(kappa, mu):
    dt = np.float32(1.0 / S_FULL)
    kdt = np.float32(np.float32(kappa) * dt)
    sc = np.empty((P, 4), np.float32)
    sc[:, 0] = kdt
    sc[:, 1] = np.float32(np.float32(1.0) - kdt)
    sc[:, 2] = np.float32(mu)
    sc[:, 3] = 0.0
    return sc


def _pretranspose_w(w_core, s_len):
    # (2048, S) row-major -> zero-interleaved time-major [P, S, 2G]:
    # out[p, t, 2g] = 0, out[p, t, 2g+1] = w[g*128+p, t]  (scan data0)
    wt = w_core.reshape(G, P, s_len).transpose(1, 2, 0)
    wz = np.zeros((P, s_len, 2 * G), np.float32)
    wz[:, :, 1::2] = wt
    return wz


def _pretranspose_x(x_core):
    return np.ascontiguousarray(x_core.reshape(G, P, L).transpose(1, 0, 2))


def _untranspose_out(o_core, s_len):
    # [P, S, G] -> (2048, S)
    return o_core.transpose(2, 0, 1).reshape(B_CORE, s_len)


def kernel(x, W, kappa, mu, sigma, _trace=False):
    x = np.ascontiguousarray(np.asarray(x, np.float32).reshape(B_FULL, L))
    W = np.ascontiguousarray(np.asarray(W, np.float32))
    kappa_v = float(np.asarray(kappa).reshape(-1)[0])
    mu_v = float(np.asarray(mu).reshape(-1)[0])
    sigma_v = np.float32(np.asarray(sigma).reshape(-1)[0])
    dt = np.float32(1.0 / S_FULL)
    sig2dt = np.float32(np.float32(sigma_v * sigma_v) * dt)
    sc = _make_sc(kappa_v, mu_v)

    nc = _get_prog(sig2dt)
    in_maps = []
    for i in range(N_CORES):
        sl = slice(i * B_CORE, (i + 1) * B_CORE)
        in_maps.append({
            "x_in": _pretranspose_x(x[sl]),
            "w_in": _pretranspose_w(W[sl], S_FULL),
            "sc_in": sc,
        })

    res = run_bass_kernel_spmd(nc, in_maps, list(range(N_CORES)), trace=_trace)
    out = np.concatenate(
        [_untranspose_out(r["out"], S_FULL) for r in res.results], axis=0
    )
    out = out.reshape(B_FULL, S_FULL, 1).astype(np.float32)
    if _trace:
        return out, res
    return out


# revision 24
# speedup vs baseline: 1.3679x; 1.0106x over previous
"""Trainium2 Bass kernel for the CIR Euler-Maruyama sampling problem.

Full inputs:  x (16384, 64, 1) f32, W (16384, 2048) f32, kappa/mu/sigma (1,) f32
Full output:  (16384, 2048, 1) f32

Strategy: pure data-parallel over batch across 8 NeuronCores (2048 rows/core).
Rows are laid out as [128 partitions x 16 free]; W / output are passed to the
device pre-transposed to time-major [128, S, 16] so every on-chip access and
DMA is contiguous.

Per-step recurrence v' = v + kappa*(m - v)*dt + sigma*sqrt(relu(v)*dt)*w is
computed as (everything fp32), with ubar = a*v + kappa*dt*m, a = 1-kappa*dt:
    sq  = Sqrt(y * (sigma^2*dt))   [ACT, immediate scale, on chain]
    v'  = scan pair: (0*st+sq), (w*sq+ubar)  [one DVE tensor_tensor_scan,
                                              W arrives zero-interleaved]
    ubar'= (v' * a) + mprime       [DVE stt, hidden under ACT latency]
    y   = max(v', 0)               [DVE tensor_tensor, on chain -> next sqrt]
The DVE->ACT->DVE sqrt round-trip plus two DVE links (~1.05us/step) dominates
the 2048-step sequential chain; per-chunk post-processing
out = 0.5*v + 0.5*xmean runs on DVE in the idle windows while waiting on ACT,
and all DMA (time-major contiguous, host-pretransposed) overlaps on the sync
engine.
"""

import numpy as np
from contextlib import ExitStack

import concourse.bass as bass
import concourse.bacc as bacc
import concourse.tile as tile
import concourse.mybir as mybir
from concourse.bass_utils import run_bass_kernel_spmd

F32 = mybir.dt.float32
AF = mybir.ActivationFunctionType
OP = mybir.AluOpType
AX = mybir.AxisListType

N_CORES = 8
B_FULL = 16384
S_FULL = 2048
L = 64
P = 128
B_CORE = B_FULL // N_CORES  # 2048
G = B_CORE // P             # 16 row-groups in the free dim

_prog_cache = {}


def _build(s_len, tc_steps, sig2dt):
    """Build + compile the per-core Bass program. sig2dt is baked as an
    immediate into the Sqrt activation's scale."""
    assert s_len % tc_steps == 0
    nchunk = s_len // tc_steps

    nc = bacc.Bacc("TRN2", target_bir_lowering=False, debug=False)

    xdr = nc.dram_tensor("x_in", [P, G, L], F32, kind="ExternalInput")
    # W arrives zero-interleaved: [..., 2g] = 0, [..., 2g+1] = w  (scan data0)
    wdr = nc.dram_tensor("w_in", [P, s_len, 2 * G], F32, kind="ExternalInput")
    scdr = nc.dram_tensor("sc_in", [P, 4], F32, kind="ExternalInput")
    odr = nc.dram_tensor("out", [P, s_len, G], F32, kind="ExternalOutput")

    with ExitStack() as ctx:
        tc = ctx.enter_context(tile.TileContext(nc))
        const = ctx.enter_context(tc.tile_pool(name="const", bufs=1))
        wpool = ctx.enter_context(tc.tile_pool(name="wpool", bufs=2))
        vpool = ctx.enter_context(tc.tile_pool(name="vpool", bufs=2))
        opool = ctx.enter_context(tc.tile_pool(name="opool", bufs=2))
        smalls = ctx.enter_context(tc.tile_pool(name="smalls", bufs=8))

        # ---- prologue: constants ----
        x_sb = const.tile([P, G, L], F32, tag="x_sb")
        nc.sync.dma_start(out=x_sb[:], in_=xdr.ap())
        sc_sb = const.tile([P, 4], F32, tag="sc_sb")
        nc.sync.dma_start(out=sc_sb[:], in_=scdr.ap())
        kdt_pp = sc_sb[:, 0:1]   # kappa*dt
        a_pp = sc_sb[:, 1:2]     # 1 - kappa*dt
        mu_pp = sc_sb[:, 2:3]    # mu

        xmr = const.tile([P, G], F32, tag="xmr")
        nc.vector.tensor_reduce(xmr[:], x_sb[:], axis=AX.X, op=OP.add)
        m = const.tile([P, G], F32, tag="m")
        nc.vector.tensor_scalar(m[:], xmr[:], 1.0 / L, mu_pp, OP.mult, OP.add)
        xm2 = const.tile([P, G], F32, tag="xm2")
        nc.vector.tensor_scalar(xm2[:], xmr[:], 0.5 / L, None, OP.mult)
        mprime = const.tile([P, G], F32, tag="mprime")
        nc.vector.tensor_scalar(mprime[:], m[:], kdt_pp, None, OP.mult)
        zero = const.tile([P, G], F32, tag="zero")
        nc.vector.memset(zero[:], 0.0)
        v0t = const.tile([P, G], F32, tag="v0")
        nc.vector.memset(v0t[:], 0.04)
        # first scan pair: [sq_0, ubar_0]
        dcur = smalls.tile([P, G, 2], F32, tag="dpair")
        nc.scalar.activation(
            dcur[:, :, 0], v0t[:], AF.Sqrt, bias=0.0, scale=sig2dt
        )
        nc.vector.scalar_tensor_tensor(
            dcur[:, :, 1], v0t[:], a_pp, mprime[:], OP.mult, OP.add
        )

        # ---- main recurrence ----
        # per step one scan over pairs (0,sq),(w,ubar):
        #   j0: state = 0*state + sq            -> sq
        #   j1: state = w*sq + ubar             -> v'
        # Post-processing of chunk c-1 is spread through chunk c's steps in
        # quarter-g pieces that fit the DVE idle window each step, so the
        # in-order DVE queue never stalls on a block of post ops at chunk
        # boundaries. Each chunk's output DMA is deferred until its posts
        # have drained (one chunk later).
        q4 = tc_steps // 4
        post_queue = []
        pending_dma = None
        for c in range(nchunk):
            wk = wpool.tile([P, tc_steps, 2 * G], F32, tag="wk")
            nc.sync.dma_start(
                out=wk[:], in_=wdr.ap()[:, c * tc_steps:(c + 1) * tc_steps, :]
            )
            vk = vpool.tile([P, tc_steps, G, 2], F32, tag="vk")
            for tau in range(tc_steps):
                vpair = vk[:, tau, :, :].rearrange("p g t -> p (g t)")
                nc.vector.tensor_tensor_scan(
                    vpair, wk[:, tau, :],
                    dcur[:, :, :].rearrange("p g t -> p (g t)"),
                    0.0, OP.mult, OP.add,
                )
                v_new = vk[:, tau, :, 1]
                dnext = smalls.tile([P, G, 2], F32, tag="dpair")
                # y first: it is chain-critical (feeds the next sqrt); the
                # ubar shadow op trails behind it in the in-order DVE queue.
                y = smalls.tile([P, G], F32, tag="y")
                nc.vector.tensor_scalar(y[:], v_new, 0.0, None, OP.max)
                nc.vector.scalar_tensor_tensor(
                    dnext[:, :, 1], v_new, a_pp, mprime[:], OP.mult, OP.add
                )
                nc.scalar.activation(
                    dnext[:, :, 0], y[:], AF.Sqrt, bias=0.0, scale=sig2dt
                )
                dcur = dnext
                if post_queue and tau % 8 == 7:
                    post_queue.pop(0)()

            if pending_dma is not None:
                # leftover posts of the pending chunk (none when the drain
                # rate matches, i.e. 64 queued == tc_steps/4 drained)
                while post_queue:
                    post_queue.pop(0)()
                pending_dma()
            ok = opool.tile([P, tc_steps, G], F32, tag="ok")
            for g in range(G):
                for h in range(2):
                    sl = slice(h * q4 * 2, (h + 1) * q4 * 2)
                    # ACT Identity fits the ~500ns Scalar idle window per
                    # step; per-partition bias carries 0.5*xmean
                    post_queue.append(
                        lambda ok=ok, vk=vk, g=g, sl=sl: nc.scalar.activation(
                            ok[:, sl, g], vk[:, sl, g, 1], AF.Identity,
                            bias=xm2[:, g:g + 1], scale=0.5,
                        )
                    )
            pending_dma = (
                lambda ok=ok, c=c: nc.sync.dma_start(
                    out=odr.ap()[:, c * tc_steps:(c + 1) * tc_steps, :],
                    in_=ok[:],
                )
            )
        # tail: drain the last chunk's posts + its DMA
        for fn in post_queue:
            fn()
        pending_dma()

    nc.compile()
    return nc


def _get_prog(sig2dt, s_len=S_FULL, tc_steps=256):
    key = (s_len, tc_steps, float(sig2dt))
    if key not in _prog_cache:
        _prog_cache[key] = _build(s_len, tc_steps, float(sig2dt))
    return _prog_cache[key]


def _make_sc(kappa, mu):
    dt = np.float32(1.0 / S_FULL)
    kdt = np.float32(np.float32(kappa) * dt)
    sc = np.empty((P, 4), np.float32)
    sc[:, 0] = kdt
    sc[:, 1] = np.float32(np.float32(1.0) - kdt)
    sc[:, 2] = np.float32(mu)
    sc[:, 3] = 0.0
    return sc


def _pretranspose_w(w_core, s_len):
    # (2048, S) row-major -> zero-interleaved time-major [P, S, 2G]:
    # out[p, t, 2g] = 0, out[p, t, 2g+1] = w[g*128+p, t]  (scan data0)
    wt = w_core.reshape(G, P, s_len).transpose(1, 2, 0)
    wz = np.zeros((P, s_len, 2 * G), np.float32)
    wz[:, :, 1::2] = wt
    return wz


def _pretranspose_x(x_core):
    return np.ascontiguousarray(x_core.reshape(G, P, L).transpose(1, 0, 2))


def _untranspose_out(o_core, s_len):
    # [P, S, G] -> (2048, S)
    return o_core.transpose(2, 0, 1).reshape(B_CORE, s_len)


def kernel(x, W, kappa, mu, sigma, _trace=False):
    x = np.ascontiguousarray(np.asarray(x, np.float32).reshape(B_FULL, L))
    W = np.ascontiguousarray(np.asarray(W, np.float32))
    kappa_v = float(np.asarray(kappa).reshape(-1)[0])
    mu_v = float(np.asarray(mu).reshape(-1)[0])
    sigma_v = np.float32(np.asarray(sigma).reshape(-1)[0])
    dt = np.float32(1.0 / S_FULL)
    sig2dt = np.float32(np.float32(sigma_v * sigma_v) * dt)
    sc = _make_sc(kappa_v, mu_v)

    nc = _get_prog(sig2dt)
    in_maps = []
    for i in range(N_CORES):
        sl = slice(i * B_CORE, (i + 1) * B_CORE)
        in_maps.append({
            "x_in": _pretranspose_x(x[sl]),
            "w_in": _pretranspose_w(W[sl], S_FULL),
            "sc_in": sc,
        })

    res = run_bass_kernel_spmd(nc, in_maps, list(range(N_CORES)), trace=_trace)
    out = np.concatenate(
        [_untranspose_out(r["out"], S_FULL) for r in res.results], axis=0
    )
    out = out.reshape(B_FULL, S_FULL, 1).astype(np.float32)
    if _trace:
        return out, res
    return out
